# revision 15
# baseline (speedup 1.0000x reference)
"""Trainium2 Bass kernel for nn_NoduleRecallLoss (segment-reduce recall loss).

Computation (matches the reference):
    fg   = x[:, 1]                    # foreground logits [B,S,S,S]
    yb   = (y > 0) as float           # binary GT
    tp[s]    = sum over voxels with comp_labels==s of fg*yb
    tp_fn[s] = sum over voxels with comp_labels==s of yb
    recall = sum_{s=1..num_components} tp[s]/tp_fn[s]
    loss   = -(recall + 1) / (num_components + 1)

Strategy: data-parallel over 8 NeuronCores (flat voxel sharding). Labels are
radix-decomposed as lab = 16*hi + lo.  The host stages w = 16*yb + lo so the
lo one-hot masks carry the yb factor for free:
    d(lo=l AND yb=1) = d(w == 16+l),    yb = d(w >= 16)
Eight 128-voxel chunks are batched into ONE LDWEIGHTS [128, 128] + ONE
MATMUL [128, 144] per group (off-diagonal products land in PSUM cells the
host ignores).  Matmul operands only allow single-stride APs, so masks and
values live in a blocked group-major layout with 8-contiguous inner runs
(measured to keep full DVE perf modes):
    wts[p, g, 8l+b] = lo-mask l of chunk 8g+b
    mov[p, g, 8j+b] = value column j of chunk 8g+b
Value columns j: [fg, hi_1..8*fg, ones, hi_1..8] where the hi-slabs are
v_h = 2*(lab>=16h)-1 for h=1..5 (Sign activations on the otherwise-idle
SCALAR engine; host recovers ge columns as (v_col + base_col)/2) and raw
ge_h for h=6..8 (DVE).  Mask building is fused into giant broadcast
tensor_tensor ops against small constant-pattern tiles: one is_equal TT
writes all 15 one-hot slabs, one is_ge TT writes 3 hi slabs, one mult TT
writes all 8 product slabs.  The PE accumulates cumulative-hi sums into two
alternating PSUM banks; the host sums per-core [128, 144] partials,
extracts the 8 diagonal [16, 18] blocks, recovers the sign/cumulative
structure by linear ops and differencing, and applies the scalar loss.
"""

import sys

sys.path.insert(0, "/opt/trn_rl_repo")

from contextlib import ExitStack

import numpy as np
import ml_dtypes

import concourse.bacc as bacc
import concourse.tile as tile
from concourse import mybir
from concourse.bass_utils import run_bass_kernel_spmd

# Problem geometry (hardcoded per spec).
B = 2
S = 192
NVOX = B * S * S * S  # 14,155,776
NCORES = 8
V8 = NVOX // NCORES  # 1,769,472 voxels per core
P = 128
FT = V8 // P  # 13,824 free columns per core
FTILE = 1152  # columns per tile
NT = FT // FTILE  # 12 tiles
G = 8  # chunks (free columns) batched per matmul
NGRP = FTILE // G  # 144 matmul groups per tile
NH = 9  # hi digit 0..8
NL = 16  # lo digit 0..15
NCOL = NH * 2  # value columns per chunk
MM = NL * G  # 128 stationary columns per group
NN = NCOL * G  # 144 moving columns per group
NSC = 5  # hi slabs built as Sign on the scalar engine (rest: DVE is_ge)
CW = (NL - 1) * G + (NH - 1 - NSC) * G  # cpat: 120 eq + 24 ge values

_BF16 = mybir.dt.bfloat16
_F32 = mybir.dt.float32
_A = mybir.AluOpType
_ACT = mybir.ActivationFunctionType


def _build_program():
    nc = bacc.Bacc("TRN2", target_bir_lowering=False)
    fgd = nc.dram_tensor("fgd", [P, NT, FTILE], _BF16, kind="ExternalInput")
    labd = nc.dram_tensor("labd", [P, NT, FTILE], _BF16, kind="ExternalInput")
    wd = nc.dram_tensor("wd", [P, NT, FTILE], _BF16, kind="ExternalInput")
    biasd = nc.dram_tensor("biasd", [P, NSC], _F32, kind="ExternalInput")
    cpatd = nc.dram_tensor("cpatd", [P, CW], _BF16, kind="ExternalInput")
    out = nc.dram_tensor("out", [P, 2, NN], _F32, kind="ExternalOutput")

    ngroups_total = NT * NGRP

    with ExitStack() as ctx:
        tc = ctx.enter_context(tile.TileContext(nc))
        work = ctx.enter_context(tc.tile_pool(name="work", bufs=2))
        psum = ctx.enter_context(tc.tile_pool(name="psum", bufs=1, space="PSUM"))
        outp = ctx.enter_context(tc.tile_pool(name="outp", bufs=1))

        acc0 = psum.tile([P, NN], _F32, tag="acc0")
        acc1 = psum.tile([P, NN], _F32, tag="acc1")
        accs = [acc0, acc1]

        biast = outp.tile([P, NSC], _F32, tag="biast")
        nc.sync.dma_start(out=biast[:], in_=biasd[:, :])
        cpat = outp.tile([P, 1, CW], _BF16, tag="cpat")
        nc.sync.dma_start(out=cpat[:, 0, :], in_=cpatd[:, :])

        for t in range(NT):
            # inputs viewed [P, NGRP, 1, G] so broadcast TT APs line up with
            # the blocked outputs (mixed dense/blocked APs drop perf modes)
            lab = work.tile([P, NGRP, 1, G], _BF16, tag="lab")
            w = work.tile([P, NGRP, 1, G], _BF16, tag="w")
            fg = work.tile([P, NGRP, 1, G], _BF16, tag="fg")
            wts = work.tile([P, NGRP, MM], _BF16, tag="wts")
            mov = work.tile([P, NGRP, NN], _BF16, tag="mov")
            nc.sync.dma_start(out=fg[:, :, 0, :], in_=fgd[:, t, :])
            nc.sync.dma_start(out=lab[:, :, 0, :], in_=labd[:, t, :])
            nc.sync.dma_start(out=w[:, :, 0, :], in_=wd[:, t, :])

            # SCALAR: fg into value column block j=0
            nc.scalar.activation(
                out=mov[:, :, 0:G], in_=fg[:, :, 0, :], func=_ACT.Copy,
            )
            # ones column j=9 (persists per pool buffer; write once per buffer)
            if t < 2:
                nc.vector.tensor_scalar(
                    out=mov[:, :, G * NH : G * NH + G], in0=w[:, :, 0, :],
                    scalar1=-1e30, scalar2=None, op0=_A.is_ge,
                )
            # SCALAR: v_h = Sign(lab - 16h + 0.5), h=1..NSC -> j=10..10+NSC-1
            for h in range(1, NSC + 1):
                j = NH + h
                nc.scalar.activation(
                    out=mov[:, :, G * j : G * j + G], in_=lab[:, :, 0, :],
                    func=_ACT.Sign, bias=biast[:, h - 1 : h], scale=1.0,
                )
            # DVE: raw ge_h for h=NSC+1..8 in one broadcast is_ge TT
            nge = NH - 1 - NSC
            nc.vector.tensor_tensor(
                out=mov[:, :, G * (NH + NSC + 1) : G * NCOL],
                in0=lab[:, :, :, :].to_broadcast([P, NGRP, nge, G]),
                in1=cpat[:, :, (NL - 1) * G : CW].to_broadcast(
                    [P, NGRP, nge * G]
                ),
                op=_A.is_ge,
            )
            # DVE: all 8 product slabs hi_h * fg in one broadcast mult TT
            nc.vector.tensor_tensor(
                out=mov[:, :, G : G * NH],
                in0=mov[:, :, G * (NH + 1) : G * NCOL],
                in1=fg[:, :, :, :].to_broadcast([P, NGRP, NH - 1, G]),
                op=_A.mult,
            )
            # DVE: yb column l=0, then all 15 one-hot slabs in one eq TT
            nc.vector.tensor_scalar(
                out=wts[:, :, 0:G], in0=w[:, :, 0, :], scalar1=16.0,
                scalar2=None, op0=_A.is_ge,
            )
            nc.vector.tensor_tensor(
                out=wts[:, :, G:MM],
                in0=w[:, :, :, :].to_broadcast([P, NGRP, NL - 1, G]),
                in1=cpat[:, :, 0 : (NL - 1) * G].to_broadcast(
                    [P, NGRP, (NL - 1) * G]
                ),
                op=_A.is_equal,
            )
            for g in range(NGRP):
                gi = t * NGRP + g
                a = accs[gi % 2]
                nc.tensor.matmul(
                    a[:],
                    lhsT=wts[:, g, :],
                    rhs=mov[:, g, :],
                    start=gi < 2,
                    stop=gi >= ngroups_total - 2,
                )
        res = outp.tile([P, 2, NN], _F32)
        nc.vector.tensor_copy(out=res[:, 0, :], in_=acc0[:])
        nc.vector.tensor_copy(out=res[:, 1, :], in_=acc1[:])
        nc.sync.dma_start(out=out[:, :, :], in_=res[:])
    if not nc.is_finalized():
        nc.finalize()
    return nc


_PROGRAM = None


def _get_program():
    global _PROGRAM
    if _PROGRAM is None:
        _PROGRAM = _build_program()
    return _PROGRAM


def make_in_maps(x, y, comp_labels):
    """Host-side sharding + dtype staging (casts/packing only)."""
    bf16 = ml_dtypes.bfloat16
    fg = np.ascontiguousarray(x[:, 1]).reshape(-1).astype(bf16)
    lab = comp_labels.reshape(-1)
    yb = (y.reshape(-1) > 0).astype(np.int32)
    w = ((yb << 4) | (lab & 15)).astype(bf16)
    labf = lab.astype(bf16)
    biases = np.tile(
        (0.5 - 16.0 * np.arange(1, NSC + 1, dtype=np.float32))[None, :], (P, 1)
    )
    # constant patterns: eq targets 17..31 (x8), ge targets 16h h=NSC+1..8 (x8)
    eqv = (17 + np.arange((NL - 1) * G) // G).astype(np.float32)
    gev = (16.0 * (NSC + 1 + np.arange((NH - 1 - NSC) * G) // G)).astype(
        np.float32
    )
    cpat = np.tile(
        np.concatenate([eqv, gev]).astype(bf16)[None, :], (P, 1)
    )
    in_maps = []
    for c in range(NCORES):
        sl = slice(c * V8, (c + 1) * V8)
        in_maps.append(
            {
                "fgd": fg[sl].reshape(P, NT, FTILE),
                "labd": labf[sl].reshape(P, NT, FTILE),
                "wd": w[sl].reshape(P, NT, FTILE),
                "biasd": biases,
                "cpatd": cpat,
            }
        )
    return in_maps


def combine_partials(outs, num_components):
    """Sum per-core [128, 2, 144] partials -> loss scalar (host all-reduce).

    PSUM cell (m, n) with m = 8l + b, n = 8j + b' is valid on the block
    diagonal b == b'.  After extraction the raw [16, 18] matrix O has
      rows   l=0: yb (sum over all lo), l=1..15: d(lo==l & yb)
      cols   j=0: fg | j=1..8: hi_j*fg | j=9: ones | j=10..17: hi_{j-9}
    where hi_h = 2*ge_h-1 for h<=NSC (Sign basis) and ge_h for h>NSC.
    Recover ge-based cumulative columns linearly, then difference.
    """
    Of = np.zeros((P, NN), np.float64)
    for o in outs:
        Of += o.astype(np.float64).sum(axis=1)
    Ob = Of.reshape(NL, G, NCOL, G)
    O = np.einsum("lbjb->lj", Ob)
    # sign basis -> ge recovery for h=1..NSC: ge_col = (v_col + base_col)/2
    O2 = O.copy()
    O2[:, 1 : NSC + 1] = (O[:, 1 : NSC + 1] + O[:, 0:1]) / 2
    O2[:, NH + 1 : NH + 1 + NSC] = (
        O[:, NH + 1 : NH + 1 + NSC] + O[:, NH : NH + 1]
    ) / 2
    O = O2
    # undo cumulative-h by differencing (append zero column)
    Ctp = np.concatenate([O[:, :NH], np.zeros((NL, 1))], axis=1)
    Cfn = np.concatenate([O[:, NH:], np.zeros((NL, 1))], axis=1)
    Tc = Ctp[:, :-1] - Ctp[:, 1:]  # [NL rows(raw), NH]
    Fc = Cfn[:, :-1] - Cfn[:, 1:]
    # undo missing row 0 (row 0 is the sum over all lo)
    T = np.zeros((NL, NH))
    F = np.zeros((NL, NH))
    T[1:, :] = Tc[1:, :]
    T[0, :] = Tc[0, :] - Tc[1:, :].sum(axis=0)
    F[1:, :] = Fc[1:, :]
    F[0, :] = Fc[0, :] - Fc[1:, :].sum(axis=0)
    tp = np.zeros(NL * NH, np.float64)
    tpfn = np.zeros(NL * NH, np.float64)
    for h in range(NH):
        for l in range(NL):
            tp[16 * h + l] = T[l, h]
            tpfn[16 * h + l] = F[l, h]
    n = int(num_components)
    with np.errstate(divide="ignore", invalid="ignore"):
        recall = np.sum(tp[1 : n + 1] / tpfn[1 : n + 1])
    loss = -(recall + 1.0) / (n + 1.0)
    return np.float32(loss)


def kernel(x, y, comp_labels, num_components):
    nc = _get_program()
    in_maps = make_in_maps(np.asarray(x), np.asarray(y), np.asarray(comp_labels))
    res = run_bass_kernel_spmd(nc, in_maps, list(range(NCORES)))
    outs = [res.results[c]["out"] for c in range(NCORES)]
    return combine_partials(outs, np.asarray(num_components))


# revision 17
# speedup vs baseline: 1.0367x; 1.0367x over previous
"""Trainium2 Bass kernel for nn_NoduleRecallLoss (segment-reduce recall loss).

Computation (matches the reference):
    fg   = x[:, 1]                    # foreground logits [B,S,S,S]
    yb   = (y > 0) as float           # binary GT
    tp[s]    = sum over voxels with comp_labels==s of fg*yb
    tp_fn[s] = sum over voxels with comp_labels==s of yb
    recall = sum_{s=1..num_components} tp[s]/tp_fn[s]
    loss   = -(recall + 1) / (num_components + 1)

Strategy: data-parallel over 8 NeuronCores (flat voxel sharding). Labels are
radix-decomposed as lab = 16*hi + lo.  The host stages w = 16*yb + lo so the
lo one-hot masks carry the yb factor for free:
    d(lo=l AND yb=1) = d(w == 16+l),    yb = d(w >= 16)
Eight 128-voxel chunks are batched into ONE LDWEIGHTS [128, 128] + ONE
MATMUL [128, 144] per group (off-diagonal products land in PSUM cells the
host ignores).  Matmul operands only allow single-stride APs, so masks and
values are built directly in a blocked layout (group-major, 8-contiguous
inner runs - measured to keep full DVE perf modes):
    wts[p, g, 8l+b] = lo-mask l of chunk 8g+b     (DVE/GPSIMD tensor_scalar)
    mov[p, g, 8j+b] = value column j of chunk 8g+b
Value columns j: [fg, v_1..8*fg, ones, v_1..8] where v_h = 2*(lab>=16h)-1
is built by the otherwise-idle SCALAR engine as Sign(lab - 16h + 0.5); the
host linearly recovers ge_h-based columns as (v_col + base_col)/2.  The PE
accumulates cumulative-hi sums the host undoes by differencing.  Two PSUM
banks alternate.  Host sums the per-core [128, 144] partials, extracts the
8 diagonal [16, 18] blocks, recovers one-hot/hi structure, applies the loss.
"""

import sys

sys.path.insert(0, "/opt/trn_rl_repo")

from contextlib import ExitStack

import numpy as np
import ml_dtypes

import concourse.bacc as bacc
import concourse.tile as tile
from concourse import mybir
from concourse.bass_utils import run_bass_kernel_spmd

# Problem geometry (hardcoded per spec).
B = 2
S = 192
NVOX = B * S * S * S  # 14,155,776
NCORES = 8
V8 = NVOX // NCORES  # 1,769,472 voxels per core
P = 128
FT = V8 // P  # 13,824 free columns per core
FTILE = 1152  # columns per tile
NT = FT // FTILE  # 12 tiles
G = 8  # chunks (free columns) batched per matmul
NGRP = FTILE // G  # 144 matmul groups per tile
NH = 9  # hi digit 0..8
NL = 16  # lo digit 0..15
NCOL = NH * 2  # value columns per chunk
MM = NL * G  # 128 stationary columns per group
NN = NCOL * G  # 144 moving columns per group
NGPS = 5  # lo-masks built on GPSIMD (rest on DVE)

_BF16 = mybir.dt.bfloat16
_F32 = mybir.dt.float32
_A = mybir.AluOpType
_ACT = mybir.ActivationFunctionType


def _build_program():
    nc = bacc.Bacc("TRN2", target_bir_lowering=False)
    fgd = nc.dram_tensor("fgd", [P, NT, FTILE], _BF16, kind="ExternalInput")
    labd = nc.dram_tensor("labd", [P, NT, FTILE], _BF16, kind="ExternalInput")
    wd = nc.dram_tensor("wd", [P, NT, FTILE], _BF16, kind="ExternalInput")
    biasd = nc.dram_tensor("biasd", [P, NH - 1], _F32, kind="ExternalInput")
    out = nc.dram_tensor("out", [P, 2, NN], _F32, kind="ExternalOutput")

    ngroups_total = NT * NGRP

    with ExitStack() as ctx:
        tc = ctx.enter_context(tile.TileContext(nc))
        work = ctx.enter_context(tc.tile_pool(name="work", bufs=2))
        psum = ctx.enter_context(tc.tile_pool(name="psum", bufs=1, space="PSUM"))
        outp = ctx.enter_context(tc.tile_pool(name="outp", bufs=1))

        acc0 = psum.tile([P, NN], _F32, tag="acc0")
        acc1 = psum.tile([P, NN], _F32, tag="acc1")
        accs = [acc0, acc1]

        biast = outp.tile([P, NH - 1], _F32, tag="biast")
        nc.sync.dma_start(out=biast[:], in_=biasd[:, :])

        for t in range(NT):
            # inputs viewed [P, NGRP, G] so DVE in/out APs match the blocked
            # outputs (measured: mixed dense-in/blocked-out drops perf mode)
            lab = work.tile([P, NGRP, G], _BF16, tag="lab")
            w = work.tile([P, NGRP, G], _BF16, tag="w")
            fg = work.tile([P, NGRP, G], _BF16, tag="fg")
            wts = work.tile([P, NGRP, MM], _BF16, tag="wts")
            mov = work.tile([P, NGRP, NN], _BF16, tag="mov")
            nc.sync.dma_start(out=fg[:, :, :], in_=fgd[:, t, :])
            nc.sync.dma_start(out=lab[:, :, :], in_=labd[:, t, :])
            nc.sync.dma_start(out=w[:, :, :], in_=wd[:, t, :])
            # GPSIMD (otherwise idle): fg into value column block j=0
            # (dense DMA + on-chip copy beats a 16B-run strided DMA dst)
            nc.gpsimd.tensor_copy(out=mov[:, :, 0:G], in_=fg[:, :, :])

            # ones column j=9 (persists per pool buffer; write once per buffer)
            if t < 2:
                nc.vector.tensor_scalar(
                    out=mov[:, :, G * NH : G * NH + G], in0=w[:, :, :],
                    scalar1=-1e30, scalar2=None, op0=_A.is_ge,
                )
            # SCALAR: v_h = Sign(lab - 16h + 0.5) = 2*(lab>=16h)-1, j=10..17
            for h in range(1, NH):
                j = NH + h
                nc.scalar.activation(
                    out=mov[:, :, G * j : G * j + G], in_=lab[:, :, :],
                    func=_ACT.Sign, bias=biast[:, h - 1 : h], scale=1.0,
                )
            # DVE first does the scalar-independent mask slabs (strict FIFO:
            # anything waiting on the scalar engine would block the queue)
            nc.vector.tensor_scalar(
                out=wts[:, :, 0:G], in0=w[:, :, :], scalar1=16.0, scalar2=None,
                op0=_A.is_ge,
            )
            for l in range(1, NL):
                nc.vector.tensor_scalar(
                    out=wts[:, :, G * l : G * l + G], in0=w[:, :, :],
                    scalar1=float(16 + l), scalar2=None, op0=_A.is_equal,
                )
            # DVE: tp value columns v_h * fg, j=1..8 (consume scalar output)
            for h in range(1, NH):
                nc.vector.tensor_tensor(
                    out=mov[:, :, G * h : G * h + G],
                    in0=mov[:, :, G * (NH + h) : G * (NH + h) + G],
                    in1=fg[:, :, :], op=_A.mult,
                )
            for g in range(NGRP):
                gi = t * NGRP + g
                a = accs[gi % 2]
                nc.tensor.matmul(
                    a[:],
                    lhsT=wts[:, g, :],
                    rhs=mov[:, g, :],
                    start=gi < 2,
                    stop=gi >= ngroups_total - 2,
                )
        res = outp.tile([P, 2, NN], _F32)
        nc.vector.tensor_copy(out=res[:, 0, :], in_=acc0[:])
        nc.vector.tensor_copy(out=res[:, 1, :], in_=acc1[:])
        nc.sync.dma_start(out=out[:, :, :], in_=res[:])
    if not nc.is_finalized():
        nc.finalize()
    return nc


_PROGRAM = None


def _get_program():
    global _PROGRAM
    if _PROGRAM is None:
        _PROGRAM = _build_program()
    return _PROGRAM


def make_in_maps(x, y, comp_labels):
    """Host-side sharding + dtype staging (casts/packing only)."""
    bf16 = ml_dtypes.bfloat16
    fg = np.ascontiguousarray(x[:, 1]).reshape(-1).astype(bf16)
    lab = comp_labels.reshape(-1)
    yb = (y.reshape(-1) > 0).astype(np.int32)
    w = ((yb << 4) | (lab & 15)).astype(bf16)
    labf = lab.astype(bf16)
    biases = np.tile(
        (0.5 - 16.0 * np.arange(1, NH, dtype=np.float32))[None, :], (P, 1)
    )
    in_maps = []
    for c in range(NCORES):
        sl = slice(c * V8, (c + 1) * V8)
        in_maps.append(
            {
                "fgd": fg[sl].reshape(P, NT, FTILE),
                "labd": labf[sl].reshape(P, NT, FTILE),
                "wd": w[sl].reshape(P, NT, FTILE),
                "biasd": biases,
            }
        )
    return in_maps


def combine_partials(outs, num_components):
    """Sum per-core [128, 2, 144] partials -> loss scalar (host all-reduce).

    PSUM cell (m, n) with m = 8l + b, n = 8j + b' is valid on the block
    diagonal b == b'.  After extraction the raw [16, 18] matrix O has
      rows   l=0: yb (sum over all lo), l=1..15: d(lo==l & yb)
      cols   j=0: fg | j=1..8: v_j*fg | j=9: ones | j=10..17: v_{j-9}
    with v_h = 2*ge_h - 1.  Recover ge-based cumulative columns linearly,
    then difference as in the radix scheme.
    """
    Of = np.zeros((P, NN), np.float64)
    for o in outs:
        Of += o.astype(np.float64).sum(axis=1)
    Ob = Of.reshape(NL, G, NCOL, G)
    O = np.einsum("lbjb->lj", Ob)
    # v -> ge recovery: ge_col = (v_col + base_col) / 2
    O2 = O.copy()
    O2[:, 1:NH] = (O[:, 1:NH] + O[:, 0:1]) / 2
    O2[:, NH + 1 :] = (O[:, NH + 1 :] + O[:, NH : NH + 1]) / 2
    O = O2
    # undo cumulative-h by differencing (append zero column)
    Ctp = np.concatenate([O[:, :NH], np.zeros((NL, 1))], axis=1)
    Cfn = np.concatenate([O[:, NH:], np.zeros((NL, 1))], axis=1)
    Tc = Ctp[:, :-1] - Ctp[:, 1:]  # [NL rows(raw), NH]
    Fc = Cfn[:, :-1] - Cfn[:, 1:]
    # undo missing row 0 (row 0 is the sum over all lo)
    T = np.zeros((NL, NH))
    F = np.zeros((NL, NH))
    T[1:, :] = Tc[1:, :]
    T[0, :] = Tc[0, :] - Tc[1:, :].sum(axis=0)
    F[1:, :] = Fc[1:, :]
    F[0, :] = Fc[0, :] - Fc[1:, :].sum(axis=0)
    tp = np.zeros(NL * NH, np.float64)
    tpfn = np.zeros(NL * NH, np.float64)
    for h in range(NH):
        for l in range(NL):
            tp[16 * h + l] = T[l, h]
            tpfn[16 * h + l] = F[l, h]
    n = int(num_components)
    with np.errstate(divide="ignore", invalid="ignore"):
        recall = np.sum(tp[1 : n + 1] / tpfn[1 : n + 1])
    loss = -(recall + 1.0) / (n + 1.0)
    return np.float32(loss)


def kernel(x, y, comp_labels, num_components):
    nc = _get_program()
    in_maps = make_in_maps(np.asarray(x), np.asarray(y), np.asarray(comp_labels))
    res = run_bass_kernel_spmd(nc, in_maps, list(range(NCORES)))
    outs = [res.results[c]["out"] for c in range(NCORES)]
    return combine_partials(outs, np.asarray(num_components))


# revision 18
# speedup vs baseline: 1.3663x; 1.3179x over previous
"""Trainium2 Bass kernel for nn_NoduleRecallLoss (segment-reduce recall loss).

Computation (matches the reference):
    fg   = x[:, 1]                    # foreground logits [B,S,S,S]
    yb   = (y > 0) as float           # binary GT
    tp[s]    = sum over voxels with comp_labels==s of fg*yb
    tp_fn[s] = sum over voxels with comp_labels==s of yb
    recall = sum_{s=1..num_components} tp[s]/tp_fn[s]
    loss   = -(recall + 1) / (num_components + 1)

Strategy: data-parallel over 8 NeuronCores (flat voxel sharding). Labels are
radix-decomposed as lab = 16*hi + lo.  The host stages w = 16*yb + lo so the
lo one-hot masks carry the yb factor for free:
    d(lo=l AND yb=1) = d(w == 16+l),    yb = d(w >= 16)
Eight 128-voxel chunks are batched into ONE LDWEIGHTS [128, 128] + ONE
MATMUL [128, 144] per group (off-diagonal products land in PSUM cells the
host ignores).  Matmul operands only allow single-stride APs, so masks and
values are built directly in a blocked layout (group-major, 8-contiguous
inner runs - measured to keep full DVE perf modes):
    wts[p, g, 8l+b] = lo-mask l of chunk 8g+b     (DVE/GPSIMD tensor_scalar)
    mov[p, g, 8j+b] = value column j of chunk 8g+b
Value columns j: [fg, v_1..8*fg, ones, v_1..8] where v_h = 2*(lab>=16h)-1
is built by the otherwise-idle SCALAR engine as Sign(lab - 16h + 0.5); the
host linearly recovers ge_h-based columns as (v_col + base_col)/2.  The PE
accumulates cumulative-hi sums the host undoes by differencing.  Two PSUM
banks alternate.  Host sums the per-core [128, 144] partials, extracts the
8 diagonal [16, 18] blocks, recovers one-hot/hi structure, applies the loss.
"""

import sys

sys.path.insert(0, "/opt/trn_rl_repo")

from contextlib import ExitStack

import numpy as np
import ml_dtypes

import concourse.bacc as bacc
import concourse.tile as tile
from concourse import mybir
from concourse.bass_utils import run_bass_kernel_spmd

# Problem geometry (hardcoded per spec).
B = 2
S = 192
NVOX = B * S * S * S  # 14,155,776
NCORES = 8
V8 = NVOX // NCORES  # 1,769,472 voxels per core
P = 128
FT = V8 // P  # 13,824 free columns per core
FTILE = 1152  # columns per tile
NT = FT // FTILE  # 12 tiles
G = 8  # chunks (free columns) batched per matmul
NGRP = FTILE // G  # 144 matmul groups per tile
NH = 9  # hi digit 0..8
NL = 16  # lo digit 0..15
NCOL = NH * 2  # value columns per chunk
MM = NL * G  # 128 stationary columns per group
NN = NCOL * G  # 144 moving columns per group
NGPS = 5  # lo-masks built on GPSIMD (rest on DVE)

_BF16 = mybir.dt.bfloat16
_F32 = mybir.dt.float32
_A = mybir.AluOpType
_ACT = mybir.ActivationFunctionType


def _build_program():
    nc = bacc.Bacc("TRN2", target_bir_lowering=False)
    fgd = nc.dram_tensor("fgd", [P, NT, FTILE], _BF16, kind="ExternalInput")
    labd = nc.dram_tensor("labd", [P, NT, FTILE], _BF16, kind="ExternalInput")
    wd = nc.dram_tensor("wd", [P, NT, FTILE], _BF16, kind="ExternalInput")
    biasd = nc.dram_tensor("biasd", [P, NH - 1], _F32, kind="ExternalInput")
    out = nc.dram_tensor("out", [P, 2, NN], _F32, kind="ExternalOutput")

    ngroups_total = NT * NGRP

    with ExitStack() as ctx:
        tc = ctx.enter_context(tile.TileContext(nc))
        work = ctx.enter_context(tc.tile_pool(name="work", bufs=2))
        psum = ctx.enter_context(tc.tile_pool(name="psum", bufs=1, space="PSUM"))
        outp = ctx.enter_context(tc.tile_pool(name="outp", bufs=1))

        acc0 = psum.tile([P, NN], _F32, tag="acc0")
        acc1 = psum.tile([P, NN], _F32, tag="acc1")
        accs = [acc0, acc1]

        biast = outp.tile([P, NH - 1], _F32, tag="biast")
        nc.sync.dma_start(out=biast[:], in_=biasd[:, :])

        for t in range(NT):
            # inputs viewed [P, NGRP, G] so DVE in/out APs match the blocked
            # outputs (measured: mixed dense-in/blocked-out drops perf mode)
            lab = work.tile([P, NGRP, G], _BF16, tag="lab")
            w = work.tile([P, NGRP, G], _BF16, tag="w")
            fg = work.tile([P, NGRP, G], _BF16, tag="fg")
            wts = work.tile([P, NGRP, MM], _BF16, tag="wts")
            mov = work.tile([P, NGRP, NN], _BF16, tag="mov")
            nc.sync.dma_start(out=fg[:, :, :], in_=fgd[:, t, :])
            nc.sync.dma_start(out=lab[:, :, :], in_=labd[:, t, :])
            nc.sync.dma_start(out=w[:, :, :], in_=wd[:, t, :])
            # SCALAR: fg into value column block j=0 (dense DMA + on-chip
            # copy beats a 16B-run strided DMA destination)
            nc.scalar.activation(
                out=mov[:, :, 0:G], in_=fg[:, :, :], func=_ACT.Copy,
            )

            # ones column j=9 (persists per pool buffer; write once per buffer)
            if t < 2:
                nc.vector.tensor_scalar(
                    out=mov[:, :, G * NH : G * NH + G], in0=w[:, :, :],
                    scalar1=-1e30, scalar2=None, op0=_A.is_ge,
                )
            # SCALAR: v_h = Sign(lab - 16h + 0.5) = 2*(lab>=16h)-1, j=10..17
            for h in range(1, NH):
                j = NH + h
                nc.scalar.activation(
                    out=mov[:, :, G * j : G * j + G], in_=lab[:, :, :],
                    func=_ACT.Sign, bias=biast[:, h - 1 : h], scale=1.0,
                )
            # DVE first does the scalar-independent mask slabs (strict FIFO:
            # anything waiting on the scalar engine would block the queue)
            nc.vector.tensor_scalar(
                out=wts[:, :, 0:G], in0=w[:, :, :], scalar1=16.0, scalar2=None,
                op0=_A.is_ge,
            )
            for l in range(1, NL):
                nc.vector.tensor_scalar(
                    out=wts[:, :, G * l : G * l + G], in0=w[:, :, :],
                    scalar1=float(16 + l), scalar2=None, op0=_A.is_equal,
                )
            # DVE: tp value columns v_h * fg, j=1..8 (consume scalar output)
            for h in range(1, NH):
                nc.vector.tensor_tensor(
                    out=mov[:, :, G * h : G * h + G],
                    in0=mov[:, :, G * (NH + h) : G * (NH + h) + G],
                    in1=fg[:, :, :], op=_A.mult,
                )
            for g in range(NGRP):
                gi = t * NGRP + g
                a = accs[gi % 2]
                nc.tensor.matmul(
                    a[:],
                    lhsT=wts[:, g, :],
                    rhs=mov[:, g, :],
                    start=gi < 2,
                    stop=gi >= ngroups_total - 2,
                )
        res = outp.tile([P, 2, NN], _F32)
        nc.vector.tensor_copy(out=res[:, 0, :], in_=acc0[:])
        nc.vector.tensor_copy(out=res[:, 1, :], in_=acc1[:])
        nc.sync.dma_start(out=out[:, :, :], in_=res[:])
    if not nc.is_finalized():
        nc.finalize()
    return nc


_PROGRAM = None


def _get_program():
    global _PROGRAM
    if _PROGRAM is None:
        _PROGRAM = _build_program()
    return _PROGRAM


def make_in_maps(x, y, comp_labels):
    """Host-side sharding + dtype staging (casts/packing only)."""
    bf16 = ml_dtypes.bfloat16
    fg = np.ascontiguousarray(x[:, 1]).reshape(-1).astype(bf16)
    lab = comp_labels.reshape(-1)
    yb = (y.reshape(-1) > 0).astype(np.int32)
    w = ((yb << 4) | (lab & 15)).astype(bf16)
    labf = lab.astype(bf16)
    biases = np.tile(
        (0.5 - 16.0 * np.arange(1, NH, dtype=np.float32))[None, :], (P, 1)
    )
    in_maps = []
    for c in range(NCORES):
        sl = slice(c * V8, (c + 1) * V8)
        in_maps.append(
            {
                "fgd": fg[sl].reshape(P, NT, FTILE),
                "labd": labf[sl].reshape(P, NT, FTILE),
                "wd": w[sl].reshape(P, NT, FTILE),
                "biasd": biases,
            }
        )
    return in_maps


def combine_partials(outs, num_components):
    """Sum per-core [128, 2, 144] partials -> loss scalar (host all-reduce).

    PSUM cell (m, n) with m = 8l + b, n = 8j + b' is valid on the block
    diagonal b == b'.  After extraction the raw [16, 18] matrix O has
      rows   l=0: yb (sum over all lo), l=1..15: d(lo==l & yb)
      cols   j=0: fg | j=1..8: v_j*fg | j=9: ones | j=10..17: v_{j-9}
    with v_h = 2*ge_h - 1.  Recover ge-based cumulative columns linearly,
    then difference as in the radix scheme.
    """
    Of = np.zeros((P, NN), np.float64)
    for o in outs:
        Of += o.astype(np.float64).sum(axis=1)
    Ob = Of.reshape(NL, G, NCOL, G)
    O = np.einsum("lbjb->lj", Ob)
    # v -> ge recovery: ge_col = (v_col + base_col) / 2
    O2 = O.copy()
    O2[:, 1:NH] = (O[:, 1:NH] + O[:, 0:1]) / 2
    O2[:, NH + 1 :] = (O[:, NH + 1 :] + O[:, NH : NH + 1]) / 2
    O = O2
    # undo cumulative-h by differencing (append zero column)
    Ctp = np.concatenate([O[:, :NH], np.zeros((NL, 1))], axis=1)
    Cfn = np.concatenate([O[:, NH:], np.zeros((NL, 1))], axis=1)
    Tc = Ctp[:, :-1] - Ctp[:, 1:]  # [NL rows(raw), NH]
    Fc = Cfn[:, :-1] - Cfn[:, 1:]
    # undo missing row 0 (row 0 is the sum over all lo)
    T = np.zeros((NL, NH))
    F = np.zeros((NL, NH))
    T[1:, :] = Tc[1:, :]
    T[0, :] = Tc[0, :] - Tc[1:, :].sum(axis=0)
    F[1:, :] = Fc[1:, :]
    F[0, :] = Fc[0, :] - Fc[1:, :].sum(axis=0)
    tp = np.zeros(NL * NH, np.float64)
    tpfn = np.zeros(NL * NH, np.float64)
    for h in range(NH):
        for l in range(NL):
            tp[16 * h + l] = T[l, h]
            tpfn[16 * h + l] = F[l, h]
    n = int(num_components)
    with np.errstate(divide="ignore", invalid="ignore"):
        recall = np.sum(tp[1 : n + 1] / tpfn[1 : n + 1])
    loss = -(recall + 1.0) / (n + 1.0)
    return np.float32(loss)


def kernel(x, y, comp_labels, num_components):
    nc = _get_program()
    in_maps = make_in_maps(np.asarray(x), np.asarray(y), np.asarray(comp_labels))
    res = run_bass_kernel_spmd(nc, in_maps, list(range(NCORES)))
    outs = [res.results[c]["out"] for c in range(NCORES)]
    return combine_partials(outs, np.asarray(num_components))
